# revision 1
# baseline (speedup 1.0000x reference)
"""GSA video block kernel for 8 TRN2 NeuronCores.

Sharding: head-parallel attention (2 heads/core) -> one AllToAll that
redistributes the RMS-normed head outputs from head-sharded to
token-sharded -> token-parallel tail (out-proj + LN2 + MLP with full
weights, 256 tokens/core).

The sequential T=512 gated-slot-attention scan is replaced by an exact
chunk-parallel formulation (C=128): intra-chunk terms via causal-masked
matmuls with per-slot decay factors, inter-chunk via carried states
K[DK,M] / V[M,DV].
"""

import os
import sys

import numpy as np
import ml_dtypes

if "/opt/trn_rl_repo" not in sys.path:
    sys.path.insert(0, "/opt/trn_rl_repo")

import concourse.bass as bass  # noqa: E402
import concourse.mybir as mybir  # noqa: E402
import concourse.tile as tile  # noqa: E402
from concourse import bacc  # noqa: E402
from concourse.bass_utils import run_bass_kernel_spmd  # noqa: E402

BF16 = mybir.dt.bfloat16
F32 = mybir.dt.float32
AF = mybir.ActivationFunctionType
ALU = mybir.AluOpType
AX = mybir.AxisListType

B, T, D = 4, 512, 1024
H, DK, DV, M = 16, 64, 64, 64
MLP = 4096
EPS = 1e-6

N_CORES = 8
C = 128                    # scan chunk length
NCH = T // C               # chunks per batch = 4
TOK = B * T                # 2048 flat tokens
TT = TOK // 128            # 16 token tiles
DT = D // 128              # 8 d tiles
MT = MLP // 128            # 32 mlp tiles
TAIL = TOK // N_CORES      # 256 tokens per core in the tail
LN8 = float(np.log(0.125))
RG = [list(range(N_CORES))]

_cache = {}


def _emit(nc, tc, io):
    x_bf, x_res = io["x_bf"], io["x_res"]
    wq, wk, wv, wf = io["wq"], io["wk"], io["wv"], io["wf"]
    bqp, bkp = io["bqp"], io["bkp"]
    bvp, bfp, b1row = io["bvp"], io["bfp"], io["b1row"]
    wo, w1, b1v, w2, b2v = io["wo"], io["w1"], io["b1v"], io["w2"], io["b2v"]
    ltriT, onescol, cmask = io["ltriT"], io["onescol"], io["cmask"]
    ident, bd128, ones_row = io["ident"], io["bd128"], io["ones_row"]
    y_out, dump = io["y_out"], io["dump"]
    P = 128

    const = tc.alloc_tile_pool(name="const", bufs=1)
    persist = tc.alloc_tile_pool(name="persist", bufs=1)
    dram = tc.alloc_tile_pool(name="dram", bufs=1, space="DRAM")

    # ---- warmup collective (prepay ncfw handshake) -----------------------
    wa_in = dram.tile([8, 128], BF16, name="wa_in")
    wa_out = dram.tile([8, 128], BF16, name="wa_out")
    nc.gpsimd.collective_compute("AllReduce", ALU.add, replica_groups=RG,
                                 ins=[wa_in.opt()], outs=[wa_out.opt()])

    # ---- constants into SBUF --------------------------------------------
    def cload(ap, shape, dt, name):
        t = const.tile(shape, dt, name=name)
        nc.sync.dma_start(t[:], ap)
        return t

    ltriT_sb = cload(ltriT.ap(), [128, 128], F32, "ltriT")
    onescol_sb = cload(onescol.ap(), [128, 1], F32, "onescol")
    cmask_sb = cload(cmask.ap(), [128, 128], BF16, "cmask")
    ident_sb = cload(ident.ap(), [128, 128], BF16, "ident")
    bd128_sb = cload(bd128.ap(), [128, 128], BF16, "bd128")
    ones_row_sb = cload(ones_row.ap(), [1, 128], BF16, "ones_row")
    bqp_sb = cload(bqp.ap(), [128, 1], F32, "bqp")
    bkp_sb = cload(bkp.ap(), [128, 1], F32, "bkp")
    bvp_sb = cload(bvp.ap(), [128, 1], F32, "bvp")
    bfp_sb = cload(bfp.ap(), [128, 1], F32, "bfp")
    b1row_sb = cload(b1row.ap(), [1, MLP], BF16, "b1row")
    b1_sb = cload(b1v.ap(), [128, MT], F32, "b1")
    eps_sb = const.tile([128, 1], F32)
    nc.vector.memset(eps_sb[:], EPS)
    ln8_sb = const.tile([128, 1], F32)
    nc.vector.memset(ln8_sb[:], LN8)

    wq_sb = const.tile([128, DT, 128], BF16)
    nc.sync.dma_start(wq_sb[:], wq.ap().rearrange("(dt p) j -> p dt j", p=P))
    wk_sb = const.tile([128, DT, 128], BF16)
    nc.sync.dma_start(wk_sb[:], wk.ap().rearrange("(dt p) j -> p dt j", p=P))
    wv_sb = const.tile([128, DT, 128], BF16)
    nc.sync.dma_start(wv_sb[:], wv.ap().rearrange("(dt p) j -> p dt j", p=P))
    wf_sb = const.tile([128, DT, 128], BF16)
    nc.sync.dma_start(wf_sb[:], wf.ap().rearrange("(dt p) j -> p dt j", p=P))
    wo_sb = const.tile([128, DT, D], BF16)
    nc.sync.dma_start(wo_sb[:], wo.ap().rearrange("(dt p) j -> p dt j", p=P))

    # ---- persistent activation tensors ----------------------------------
    qT = persist.tile([128, TOK], BF16, name="qT")       # [2h*64 dk, t]
    kT = persist.tile([128, TOK], BF16, name="kT")
    k_tm = persist.tile([128, TT, 128], BF16, name="k_tm")   # [t, 2h*64]
    v_tm = persist.tile([128, TT, 128], BF16, name="v_tm")
    vT = persist.tile([128, TOK], BF16, name="vT")
    fT = persist.tile([128, TOK], BF16, name="fT")
    f_tm = persist.tile([128, TT, 128], BF16, name="f_tm")
    sp = persist.tile([128, TT, 128], F32, name="sp")        # softplus(-f)
    s_tm = persist.tile([128, TT, 128], BF16, name="s_tm")   # 1-exp(g)
    onT = persist.tile([128, TOK], BF16, name="onT")         # normed oT

    h_dram = [dram.tile([TOK // 2, D], BF16, name=f"h_dram{g}")
              for g in range(2)]
    k_dram = dram.tile([128, TOK], BF16, name="k_dram")
    v_dram = dram.tile([128, TOK], BF16, name="v_dram")
    f_dram = dram.tile([128, TOK], BF16, name="f_dram")
    z_dram = dram.tile([TAIL, MLP], BF16, name="z_dram")
    a2a_in = dram.tile([128 * N_CORES, TAIL], BF16, name="a2a_in")
    a2a_out = dram.tile([128 * N_CORES, TAIL], BF16, name="a2a_out")
    h2d = dram.tile([TAIL, D], BF16, name="h2d")

    # =====================================================================
    # P1: LN1 stats (batched sqrt) + normalize, store h to DRAM
    # =====================================================================
    p0 = tc.alloc_tile_pool(name="p0", bufs=1)
    x_sb = p0.tile([128, TT, D], BF16, name="x_sb")
    stats = p0.tile([128, 2, TT], F32, name="stats")  # [.,0,:]=r [.,1,:]=nrmu
    with tc.tile_pool(name="p1", bufs=3) as p1, \
         tc.tile_pool(name="p1s", bufs=2) as p1s:
        musb = p1.tile([128, TT], F32, name="musb")
        sssb = p1.tile([128, TT], F32, name="sssb")
        for tt in range(TT):
            nc.sync.dma_start(
                x_sb[:, tt, :],
                x_bf.ap().rearrange("(n p) d -> n p d", p=P)[tt])
            nc.vector.tensor_reduce(musb[:, tt:tt + 1], x_sb[:, tt, :],
                                    AX.X, ALU.add)
            sq = p1s.tile([128, D], BF16, name="sq")
            nc.scalar.activation(sq[:], x_sb[:, tt, :], AF.Square,
                                 accum_out=sssb[:, tt:tt + 1])
        mu = p1.tile([128, TT], F32, name="mu")
        nc.vector.tensor_scalar_mul(mu[:], musb[:], 1.0 / D)
        var = p1.tile([128, TT], F32, name="var")
        nc.vector.tensor_tensor(var[:], mu[:], mu[:], ALU.mult)
        ex2 = p1.tile([128, TT], F32, name="ex2")
        nc.vector.tensor_scalar_mul(ex2[:], sssb[:], 1.0 / D)
        nc.vector.tensor_tensor(var[:], ex2[:], var[:], ALU.subtract)
        sd = p1.tile([128, TT], F32, name="sd")
        nc.scalar.activation(sd[:], var[:], AF.Sqrt, bias=eps_sb[:])
        nc.vector.reciprocal(stats[:, 0, :], sd[:])
        nc.vector.tensor_tensor(stats[:, 1, :], stats[:, 0, :], mu[:],
                                ALU.mult)
        nc.vector.tensor_scalar_mul(stats[:, 1, :], stats[:, 1, :], -1.0)
        for tt in range(TT):
            g, i = tt // (TT // 2), tt % (TT // 2)
            ht = p1.tile([128, D], BF16, name="ht")
            nc.scalar.activation(ht[:], x_sb[:, tt, :], AF.Identity,
                                 bias=stats[:, 1, tt:tt + 1],
                                 scale=stats[:, 0, tt:tt + 1])
            nc.sync.dma_start(
                h_dram[g][:].rearrange("(n p) d -> n p d", p=P)[i], ht[:])

    p0.release()
    # =====================================================================
    # P2: transpose-load hT, projections
    # =====================================================================
    p2h = tc.alloc_tile_pool(name="p2h", bufs=1)
    hT = p2h.tile([128, DT, TOK], BF16, name="hT")
    for g in range(2):
        for dt in range(DT):
            nc.sync.dma_start_transpose(
                hT[:, dt, g * (TOK // 2):(g + 1) * (TOK // 2)],
                h_dram[g][:, dt * 128:(dt + 1) * 128])

    with tc.tile_pool(name="proj_ps", bufs=2, space="PSUM") as pps:
        for tci in range(4):
            tcsl = slice(tci * 512, (tci + 1) * 512)
            for (dst, w_sb, bias, fn) in (
                    (qT, wq_sb, bqp_sb, AF.Silu),
                    (kT, wk_sb, bkp_sb, AF.Silu),
                    (vT, wv_sb, bvp_sb, AF.Identity),
                    (fT, wf_sb, bfp_sb, AF.Identity)):
                bank = pps.tile([128, 512], F32, name="projbank")
                for dt in range(DT):
                    nc.tensor.matmul(bank[:], w_sb[:, dt, :], hT[:, dt, tcsl],
                                     start=(dt == 0), stop=(dt == DT - 1))
                nc.scalar.activation(dst[:, tcsl], bank[:], fn,
                                     bias=bias[:], scale=1.0)

    p2h.release()
    # roundtrip k/v/f to token-major via DMA transpose
    nc.sync.dma_start(k_dram[:], kT[:])
    nc.sync.dma_start(v_dram[:], vT[:])
    nc.sync.dma_start(f_dram[:], fT[:])
    for tt in range(TT):
        ttsl = slice(tt * 128, (tt + 1) * 128)
        nc.sync.dma_start_transpose(k_tm[:, tt, :], k_dram[:, ttsl])
        nc.sync.dma_start_transpose(v_tm[:, tt, :], v_dram[:, ttsl])
        nc.sync.dma_start_transpose(f_tm[:, tt, :], f_dram[:, ttsl])

    with tc.tile_pool(name="sgate", bufs=2) as sg:
        for tci in range(4):
            csl = slice(tci * 4, (tci + 1) * 4)
            enf = sg.tile([128, 4, 128], F32, name="enf")
            nc.scalar.activation(enf[:], f_tm[:, csl, :], AF.Exp,
                                 scale=-1.0)
            nc.scalar.activation(sp[:, csl, :], enf[:], AF.Ln, bias=1.0)
            e8 = sg.tile([128, 4, 128], BF16, name="e8")
            nc.scalar.activation(e8[:], sp[:, csl, :], AF.Exp, scale=-0.125)
            nc.vector.tensor_scalar(s_tm[:, csl, :],
                                    e8[:], -1.0, 1.0, ALU.mult, ALU.add)

    for nm, t_sb in (("qT", qT), ("kT", kT)):
        if (d := dump(nm, [128, TOK], BF16)) is not None:
            nc.sync.dma_start(d.ap(), t_sb[:])
    for nm, t_sb in (("k_tm", k_tm), ("v_tm", v_tm), ("s_tm", s_tm)):
        if (d := dump(nm, [128, TT * 128], BF16)) is not None:
            nc.sync.dma_start(d.ap().rearrange("p (n f) -> p n f", n=TT),
                              t_sb[:])
    if (d := dump("sp", [128, TT * 128])) is not None:
        nc.sync.dma_start(d.ap().rearrange("p (n f) -> p n f", n=TT), sp[:])

    # =====================================================================
    # P3: chunked scan, b-major; RMS + write o_nT
    # =====================================================================
    with tc.tile_pool(name="scan_ps", bufs=2, space="PSUM") as sps, \
         tc.tile_pool(name="scan_sb", bufs=2) as ssb, \
         tc.tile_pool(name="state_sb", bufs=1) as stb:
        Kst = stb.tile([128, 64], BF16, name="Kst")   # [2h*64 dk, s]
        Vst = stb.tile([128, 64], BF16, name="Vst")   # [2h*64 s, dv]
        for b in range(B):
            for c in range(NCH):
                bi = b * 4 + c
                tsl = slice(b * 512 + c * 128, b * 512 + (c + 1) * 128)
                first = (c == 0)
                bankA = sps.tile([128, 512], F32, name="bankA", bufs=1)
                ps_b = bankA[:, 0:128]
                ps_ss = bankA[:, 128:256]
                ps_lc = bankA[:, 256:257]
                ps_lcr = bankA[0:1, 257:385]
                ps_lambc = bankA[:, 384:512]
                bankB = sps.tile([128, 512], F32, name="bankB")
                ps_a = (bankB[:, 0:128], bankB[:, 128:256])
                ps_ok = bankB[:, 256:384]
                bankD = sps.tile([128, 1024], BF16, name="bankD", bufs=1)
                ps_pt = (bankD[0:64, 0:128], bankD[0:64, 128:256])
                ps_st = (bankD[0:64, 256:384], bankD[0:64, 384:512])
                bankE = sps.tile([128, 512], F32, name="bankE", bufs=1)
                ps_b2 = (bankE[:, 0:128], bankE[:, 128:256])
                bankF = sps.tile([128, 512], F32, name="bankF")
                ps_o = (bankF[0:64, 0:128], bankF[0:64, 128:256])
                ps_dk = (bankF[0:64, 256:320], bankF[0:64, 320:384])
                ps_dv = (bankF[0:64, 384:448], bankF[0:64, 448:512])

                # cumsum b = ltriT.T @ sp (f32); colsum -> bCT
                nc.tensor.matmul(ps_b, ltriT_sb[:], sp[:, bi, :],
                                 start=True, stop=True)
                nc.tensor.matmul(ps_lc, sp[:, bi, :], onescol_sb[:],
                                 start=True, stop=True)
                lam = ssb.tile([128, 128], BF16, name="lam")
                nc.scalar.activation(lam[:], ps_b, AF.Exp)
                lam_s = ssb.tile([128, 128], BF16, name="lam_s")
                nc.vector.tensor_scalar_mul(lam_s[:], lam[:], 0.125)
                en = ssb.tile([128, 128], BF16, name="en")
                with nc.allow_low_precision(reason="en=1/lam feeds bf16"):
                    nc.vector.reciprocal(en[:], lam[:])
                lamCT = ssb.tile([128, 1], F32, name="lamCT")
                nc.scalar.activation(lamCT[:], ps_lc, AF.Exp)
                nc.tensor.matmul(ps_lcr, onescol_sb[:], sp[:, bi, :],
                                 start=True, stop=True)
                lamCr = ssb.tile([1, 128], BF16, name="lamCr")
                nc.scalar.activation(lamCr[:], ps_lcr, AF.Exp)

                s_til = ssb.tile([128, 128], BF16, name="s_til")
                nc.vector.tensor_tensor(s_til[:], s_tm[:, bi, :], en[:],
                                        ALU.mult)
                nc.tensor.matmul(ps_lambc, ones_row_sb[:], lamCr[:],
                                 start=True, stop=True)
                s2 = ssb.tile([128, 128], BF16, name="s2")
                nc.vector.tensor_tensor(s2[:], s_til[:], ps_lambc, ALU.mult)

                am = ssb.tile([128, 256], BF16, name="am")
                for h in range(2):
                    hs = slice(h * 64, (h + 1) * 64)
                    nc.tensor.matmul(ps_a[h], kT[hs, tsl], qT[hs, tsl],
                                     start=True, stop=True)
                    nc.vector.tensor_tensor(am[:, h * 128:(h + 1) * 128],
                                            ps_a[h], cmask_sb[:], ALU.mult)
                for h in range(2):
                    hs = slice(h * 64, (h + 1) * 64)
                    oks = ps_ok[:, h * 64:(h + 1) * 64]
                    if not first:
                        nc.tensor.matmul(oks, qT[hs, tsl], Kst[hs, :],
                                         start=True, stop=False)
                    nc.tensor.matmul(oks, am[:, h * 128:(h + 1) * 128],
                                     s_til[:, h * 64:(h + 1) * 64],
                                     start=first, stop=True)
                # softmax over slots (per head), pl = P * lam
                oksc = ssb.tile([128, 128], F32, name="oksc")
                nc.vector.tensor_tensor(oksc[:], ps_ok, lam_s[:], ALU.mult)
                ex = ssb.tile([128, 128], BF16, name="ex")
                nc.scalar.activation(ex[:], oksc[:], AF.Exp)
                rsum = ssb.tile([128, 2], F32, name="rsum")
                nc.vector.tensor_reduce(
                    rsum[:], ex[:].rearrange("p (h s) -> p h s", h=2),
                    AX.X, ALU.add)
                rcp = ssb.tile([128, 2], F32, name="rcp")
                nc.vector.reciprocal(rcp[:], rsum[:])
                pl = ssb.tile([128, 128], BF16, name="pl")
                nc.vector.tensor_tensor(pl[:], ex[:], lam[:], ALU.mult)
                nc.vector.tensor_tensor(
                    pl[:].rearrange("p (h s) -> p h s", h=2),
                    pl[:].rearrange("p (h s) -> p h s", h=2),
                    rcp[:].rearrange("p (h o) -> p h o", h=2)
                        .to_broadcast([128, 2, 64]),
                    ALU.mult)

                # transposes: plT, s_tilT  [2h*64 s, 128 t]
                plT = ssb.tile([128, 128], BF16, name="plT")
                s_tilT = ssb.tile([128, 128], BF16, name="s_tilT")
                for h in range(2):
                    hs = slice(h * 64, (h + 1) * 64)
                    nc.tensor.transpose(ps_pt[h], pl[:, hs], ident_sb[:])
                    nc.vector.tensor_copy(plT[hs, :], ps_pt[h])
                    nc.tensor.transpose(ps_st[h], s_til[:, hs], ident_sb[:])
                    nc.vector.tensor_copy(s_tilT[hs, :], ps_st[h])

                b2m = ssb.tile([128, 256], BF16, name="b2m")
                for h in range(2):
                    hs = slice(h * 64, (h + 1) * 64)
                    nc.tensor.matmul(ps_b2[h], s_tilT[hs, :], plT[hs, :],
                                     start=True, stop=True)
                    nc.vector.tensor_tensor(b2m[:, h * 128:(h + 1) * 128],
                                            ps_b2[h], cmask_sb[:], ALU.mult)
                for h in range(2):
                    hs = slice(h * 64, (h + 1) * 64)
                    if not first:
                        nc.tensor.matmul(ps_o[h], Vst[hs, :], plT[hs, :],
                                         start=True, stop=False)
                    nc.tensor.matmul(ps_o[h], v_tm[:, bi, hs],
                                     b2m[:, h * 128:(h + 1) * 128],
                                     start=first, stop=True)
                    nc.tensor.matmul(ps_dk[h], k_tm[:, bi, hs],
                                     s2[:, hs], start=True, stop=True)
                    nc.tensor.matmul(ps_dv[h], s2[:, hs], v_tm[:, bi, hs],
                                     start=True, stop=True)
                    if first:
                        nc.vector.tensor_copy(Kst[hs, :], ps_dk[h])
                        nc.vector.tensor_copy(Vst[hs, :], ps_dv[h])
                    else:
                        nc.vector.tensor_tensor(
                            Kst[hs, :], Kst[hs, :],
                            ps_lambc[hs, hs], ALU.mult)
                        nc.vector.tensor_tensor(Kst[hs, :], Kst[hs, :],
                                                ps_dk[h], ALU.add)
                        nc.vector.tensor_scalar(Vst[hs, :], Vst[hs, :],
                                                lamCT[hs, 0:1], None, ALU.mult)
                        nc.vector.tensor_tensor(Vst[hs, :], Vst[hs, :],
                                                ps_dv[h], ALU.add)

                # write raw oT (RMS batched after the loop)
                nc.vector.tensor_copy(onT[0:64, tsl], ps_o[0])
                nc.vector.tensor_copy(onT[64:128, tsl], ps_o[1])

    # batched RMS over dv for the whole oT
    with tc.tile_pool(name="rms_ps", bufs=2, space="PSUM") as rps, \
         tc.tile_pool(name="rms_sb", bufs=2) as rsb:
        for q4 in range(4):
            qsl = slice(q4 * 512, (q4 + 1) * 512)
            sqo = rsb.tile([128, 512], BF16, name="sqo")
            nc.vector.tensor_tensor(sqo[:], onT[:, qsl], onT[:, qsl],
                                    ALU.mult)
            ps_ss = rps.tile([128, 512], F32, name="ps_ss")
            nc.tensor.matmul(ps_ss[:], bd128_sb[:], sqo[:],
                             start=True, stop=True)
            sdo = rsb.tile([128, 512], F32, name="sdo")
            nc.scalar.activation(sdo[:], ps_ss[:], AF.Sqrt,
                                 bias=eps_sb[:], scale=1.0 / DV)
            rro = rsb.tile([128, 512], F32, name="rro")
            nc.vector.reciprocal(rro[:], sdo[:])
            nc.vector.tensor_tensor(onT[:, qsl], onT[:, qsl], rro[:],
                                    ALU.mult)

    if (d := dump("onT", [128, TOK], BF16)) is not None:
        nc.sync.dma_start(d.ap(), onT[:])

    # head-sharded -> token-sharded redistribution
    nc.sync.dma_start(
        a2a_in[:].rearrange("(r p) t -> p r t", p=P),
        onT[:].rearrange("p (r t) -> p r t", r=N_CORES))
    nc.gpsimd.collective_compute("AllToAll", ALU.bypass, replica_groups=RG,
                                 ins=[a2a_in.opt()], outs=[a2a_out.opt()])

    # =====================================================================
    # P4 tail: out-proj + residual + LN2 + MLP on 256 tokens
    # =====================================================================
    with tc.tile_pool(name="tail_ps", bufs=1, space="PSUM") as tps, \
         tc.tile_pool(name="tail_sb", bufs=2) as tsb, \
         tc.tile_pool(name="tail_keep", bufs=1) as tkb, \
         tc.tile_pool(name="w1stream", bufs=3) as w1s, \
         tc.tile_pool(name="w2stream", bufs=4) as w2s:
        ofT = tkb.tile([128, DT, TAIL], BF16, name="ofT")
        nc.sync.dma_start(ofT[:],
                          a2a_out[:].rearrange("(jt p) t -> p jt t", p=P))
        x2 = tkb.tile([128, 2, D], F32, name="x2")
        nc.sync.dma_start(x2[:],
                          x_res.ap().rearrange("(n p) d -> p n d", p=P))

        op_bank = tps.tile([128, 512], F32, name="op_bank")
        for tt2 in range(2):
            for nb in range(2):
                nsl = slice(nb * 512, (nb + 1) * 512)
                for jt in range(DT):
                    nc.tensor.matmul(op_bank[:],
                                     ofT[:, jt, tt2 * 128:(tt2 + 1) * 128],
                                     wo_sb[:, jt, nsl],
                                     start=(jt == 0), stop=(jt == DT - 1))
                nc.vector.tensor_tensor(x2[:, tt2, nsl], op_bank[:],
                                        x2[:, tt2, nsl], ALU.add)
        if (d := dump("x2", [128, 2 * D])) is not None:
            nc.sync.dma_start(d.ap().rearrange("p (n f) -> p n f", n=2),
                              x2[:])

        # LN2 + store h2, transpose-load
        h2T = tkb.tile([128, DT, TAIL], BF16, name="h2T")
        for tt2 in range(2):
            x2t = x2[:, tt2, :]
            ssum = tsb.tile([128, 1], F32, name="ssum2")
            nc.vector.tensor_reduce(ssum[:], x2t, AX.X, ALU.add)
            sq = tsb.tile([128, D], BF16, name="sq2")
            ssq = tsb.tile([128, 1], F32, name="ssq2")
            nc.scalar.activation(sq[:], x2t, AF.Square, accum_out=ssq[:])
            mu = tsb.tile([128, 1], F32, name="mu2")
            nc.vector.tensor_scalar_mul(mu[:], ssum[:], 1.0 / D)
            var = tsb.tile([128, 1], F32, name="var2")
            nc.vector.tensor_tensor(var[:], mu[:], mu[:], ALU.mult)
            ex2 = tsb.tile([128, 1], F32, name="ex22")
            nc.vector.tensor_scalar_mul(ex2[:], ssq[:], 1.0 / D)
            nc.vector.tensor_tensor(var[:], ex2[:], var[:], ALU.subtract)
            sd = tsb.tile([128, 1], F32, name="sd2")
            nc.scalar.activation(sd[:], var[:], AF.Sqrt, bias=eps_sb[:])
            r2 = tsb.tile([128, 1], F32, name="r2")
            nc.vector.reciprocal(r2[:], sd[:])
            nrmu = tsb.tile([128, 1], F32, name="nrmu2")
            nc.vector.tensor_tensor(nrmu[:], r2[:], mu[:], ALU.mult)
            nc.vector.tensor_scalar_mul(nrmu[:], nrmu[:], -1.0)
            h2t = tsb.tile([128, D], BF16, name="h2t")
            nc.scalar.activation(h2t[:], x2t, AF.Identity,
                                 bias=nrmu[:], scale=r2[:])
            nc.sync.dma_start(
                h2d[:].rearrange("(n p) d -> n p d", p=P)[tt2], h2t[:])
        for dt in range(DT):
            nc.sync.dma_start_transpose(h2T[:, dt, :],
                                        h2d[:, dt * 128:(dt + 1) * 128])

        # MLP1: y1 token-major [t, mlp-chunk], gelu, roundtrip to zT
        z_tm = tkb.tile([128, 2, MLP], BF16, name="z_tm")
        for mc in range(8):
            mcsl = slice(mc * 512, (mc + 1) * 512)
            w1t = w1s.tile([128, DT, 512], BF16, name="w1t")
            nc.sync.dma_start(
                w1t[:], w1.ap().rearrange("(dt p) m -> p dt m", p=P)
                [:, :, mcsl])
            for tt2 in range(2):
                y1b = tps.tile([128, 512], F32, name="y1b", bufs=2)
                for dt in range(DT):
                    nc.tensor.matmul(y1b[:],
                                     h2T[:, dt, tt2 * 128:(tt2 + 1) * 128],
                                     w1t[:, dt, :],
                                     start=(dt == 0), stop=False)
                nc.tensor.matmul(y1b[:], ones_row_sb[:], b1row_sb[:, mcsl],
                                 start=False, stop=True)
                nc.scalar.activation(z_tm[:, tt2, mcsl], y1b[:], AF.Gelu)
        nc.sync.dma_start(
            z_dram[:].rearrange("(n p) m -> p n m", p=P), z_tm[:])
        zT = tkb.tile([128, MT, TAIL], BF16, name="zT")
        for mt in range(MT):
            nc.sync.dma_start_transpose(
                zT[:, mt, :], z_dram[:, mt * 128:(mt + 1) * 128])

        # MLP2: y2 = z @ w2, accumulate over mt into 4 resident banks
        y2_banks = [tps.tile([128, 512], F32, name=f"y2b{i}")
                    for i in range(4)]
        for mt in range(MT):
            w2t = w2s.tile([128, D], BF16, name="w2t")
            nc.sync.dma_start(
                w2t[:], w2.ap().rearrange("(n p) d -> n p d", p=P)[mt])
            for tt2 in range(2):
                for nb in range(2):
                    nc.tensor.matmul(
                        y2_banks[tt2 * 2 + nb],
                        zT[:, mt, tt2 * 128:(tt2 + 1) * 128],
                        w2t[:, nb * 512:(nb + 1) * 512],
                        start=(mt == 0), stop=(mt == MT - 1))
        for tt2 in range(2):
            for nb in range(2):
                nsl = slice(nb * 512, (nb + 1) * 512)
                ys = tsb.tile([128, 512], F32, name="ys")
                nc.vector.tensor_tensor(ys[:], y2_banks[tt2 * 2 + nb],
                                        x2[:, tt2, nsl], ALU.add)
                nc.sync.dma_start(
                    y_out.ap().rearrange("(n p) d -> p n d", p=P)
                    [:, tt2, nsl], ys[:])

    for pool in (dram, persist, const):
        pool.release()


def _build():
    nc = bacc.Bacc("TRN2", target_bir_lowering=False, debug=False,
                   num_devices=N_CORES)

    def din(name, shape, dt=BF16):
        return nc.dram_tensor(name, shape, dt, kind="ExternalInput")

    io = dict(
        x_bf=din("x_bf", [TOK, D]),
        x_res=din("x_res", [TAIL, D], F32),
        wq=din("wq", [D, 128]), wk=din("wk", [D, 128]),
        wv=din("wv", [D, 128]), wf=din("wf", [D, 128]),
        bqp=din("bqp", [128, 1], F32), bkp=din("bkp", [128, 1], F32),
        bvp=din("bvp", [128, 1], F32), bfp=din("bfp", [128, 1], F32),
        b1row=din("b1row", [1, MLP]),
        wo=din("wo", [D, D]),
        w1=din("w1", [D, MLP]),
        b1v=din("b1v", [128, MLP // 128], F32),
        w2=din("w2", [MLP, D]),
        b2v=din("b2v", [1, D], F32),
        ltriT=din("ltriT", [128, 128], F32),
        onescol=din("onescol", [128, 1], F32),
        cmask=din("cmask", [128, 128]),
        ident=din("ident", [128, 128]),
        bd128=din("bd128", [128, 128]),
        ones_row=din("ones_row", [1, 128]),
        y_out=nc.dram_tensor("y_out", [TAIL, D], F32, kind="ExternalOutput"),
    )

    dbg = [s for s in os.environ.get("GSA_DEBUG", "").split(",") if s]
    dbg_outs = {}

    def dump(name, shape, dt=F32):
        if name in dbg:
            t = nc.dram_tensor("dbg_" + name, shape, dt,
                               kind="ExternalOutput")
            dbg_outs[name] = t
            return t
        return None

    io["dump"] = dump
    with tile.TileContext(nc) as tcx:
        _emit(nc, tcx, io)
    nc.compile()
    return nc, sorted(dbg_outs)


def _host_prep(inputs):
    """Fold norms/biases into weights; build per-core in_maps."""
    f32 = np.float32
    bf16 = ml_dtypes.bfloat16
    x = np.asarray(inputs["hidden_states"], f32).reshape(TOK, D)
    ln1_w = np.asarray(inputs["ln1_w"], f32)
    ln1_b = np.asarray(inputs["ln1_b"], f32)
    ln2_w = np.asarray(inputs["ln2_w"], f32)
    ln2_b = np.asarray(inputs["ln2_b"], f32)
    gnorm = np.asarray(inputs["gnorm_w"], f32)
    Wq = np.asarray(inputs["Wq"], f32) * ln1_w[:, None]
    Wk = np.asarray(inputs["Wk"], f32) * ln1_w[:, None]
    Wv = np.asarray(inputs["Wv"], f32) * ln1_w[:, None]
    Wf = np.asarray(inputs["Wf"], f32) * ln1_w[:, None]
    bq = ln1_b @ np.asarray(inputs["Wq"], f32)
    bk = ln1_b @ np.asarray(inputs["Wk"], f32)
    bv = ln1_b @ np.asarray(inputs["Wv"], f32)
    bf_ = ln1_b @ np.asarray(inputs["Wf"], f32)
    Wo = np.asarray(inputs["Wo"], f32) * np.tile(gnorm, H)[:, None]
    W1 = np.asarray(inputs["W1"], f32) * ln2_w[:, None]
    b1 = np.asarray(inputs["b1"], f32) + ln2_b @ np.asarray(inputs["W1"], f32)
    W2 = np.asarray(inputs["W2"], f32)
    b2 = np.asarray(inputs["b2"], f32)

    tri = np.tril(np.ones((128, 128), f32))  # [t, tau] tau<=t
    ltriT = np.ascontiguousarray((-0.125 * tri).T)           # [tau, t]
    cmask = np.ascontiguousarray(tri.T.astype(bf16))         # [tau, t]
    ident = np.eye(128, dtype=bf16)
    bd128 = np.kron(np.eye(2, dtype=f32),
                    np.ones((64, 64), f32)).astype(bf16)
    common = dict(
        x_bf=np.ascontiguousarray(x.astype(bf16)),
        ltriT=ltriT,
        onescol=np.full((128, 1), -0.125, f32),
        cmask=cmask, ident=ident, bd128=bd128,
        ones_row=np.ones((1, 128), bf16),
        wo=np.ascontiguousarray(Wo.astype(bf16)),
        w1=np.ascontiguousarray(W1.astype(bf16)),
        b1v=np.ascontiguousarray(b1.reshape(MLP // 128, 128).T.astype(f32)),
        b1row=np.ascontiguousarray(b1.reshape(1, MLP).astype(bf16)),
        w2=np.ascontiguousarray(W2.astype(bf16)),
        b2v=np.ascontiguousarray(b2.reshape(1, D)),
    )
    in_maps = []
    for r in range(N_CORES):
        jsl = slice(r * 128, (r + 1) * 128)  # 2 heads = 128 cols
        m = dict(common)
        m["x_res"] = np.ascontiguousarray(x[r * TAIL:(r + 1) * TAIL]
                                          + b2[None, :])
        m["wq"] = np.ascontiguousarray(Wq[:, jsl].astype(bf16))
        m["wk"] = np.ascontiguousarray(Wk[:, jsl].astype(bf16))
        m["wv"] = np.ascontiguousarray(Wv[:, jsl].astype(bf16))
        m["wf"] = np.ascontiguousarray(Wf[:, jsl].astype(bf16))
        m["bqp"] = np.ascontiguousarray(bq[jsl].reshape(128, 1))
        m["bkp"] = np.ascontiguousarray(bk[jsl].reshape(128, 1))
        m["bvp"] = np.ascontiguousarray(bv[jsl].reshape(128, 1))
        m["bfp"] = np.ascontiguousarray(bf_[jsl].reshape(128, 1))
        in_maps.append(m)
    return in_maps


def kernel(**inputs):
    if "nc" not in _cache:
        _cache["nc"], _cache["dbg"] = _build()
    nc = _cache["nc"]
    in_maps = _host_prep(inputs)
    res = run_bass_kernel_spmd(nc, in_maps, core_ids=list(range(N_CORES)),
                               trace=bool(os.environ.get("GSA_TRACE")))
    _cache["last_results"] = res
    out = np.concatenate([res.results[r]["y_out"] for r in range(N_CORES)],
                         axis=0)
    return out.reshape(B, T, D)



# revision 19
# speedup vs baseline: 1.7309x; 1.7309x over previous
"""GSA video block kernel for 8 TRN2 NeuronCores (v2).

Sharding: head-parallel attention (2 heads/core) -> one AllToAll that
redistributes the RMS-normed head outputs from head-sharded to
token-sharded -> token-parallel tail (out-proj + LN2 + MLP with full
weights, 256 tokens/core).

v2 layout strategy: the host supplies x TRANSPOSED (xT [D, TOK]), so
LN1 never materializes h -- projections run feat-major from xT with a
rank-1 mean correction accumulated into the psum and a broadcast-rstd
multiply afterwards.  Token-major k/v/f come from tensor-engine
transposes (no DMA transposes anywhere).  The chunked scan processes
the 4 batches' chunk-c tiles as single [128,512]-wide ops, with all
state-independent prep emitted ahead of the serial state chain.  The
MLP computes y1 m-major so MLP2 needs no transpose.
"""

import os
import sys

import numpy as np
import ml_dtypes

if "/opt/trn_rl_repo" not in sys.path:
    sys.path.insert(0, "/opt/trn_rl_repo")

import concourse.bass as bass  # noqa: E402
import concourse.mybir as mybir  # noqa: E402
import concourse.tile as tile  # noqa: E402
from concourse import bacc  # noqa: E402
from concourse.bass_utils import run_bass_kernel_spmd  # noqa: E402

BF16 = mybir.dt.bfloat16
F32 = mybir.dt.float32
AF = mybir.ActivationFunctionType
ALU = mybir.AluOpType
AX = mybir.AxisListType

B, T, D = 4, 512, 1024
H, DK, DV, M = 16, 64, 64, 64
MLP = 4096
EPS = 1e-6

N_CORES = 8
C = 128                    # scan chunk length
NCH = T // C               # chunks per batch = 4
TOK = B * T                # 2048 flat tokens
TT = TOK // 128            # 16 token tiles
DT = D // 128              # 8 d tiles
MT = MLP // 128            # 32 mlp tiles
TAIL = TOK // N_CORES      # 256 tokens per core in the tail
RG = [list(range(N_CORES))]
P = 128

_cache = {}


def _emit(nc, tc, io):
    xT, x_res = io["xT"], io["x_res"]
    wqkvf, wo, w1, w2 = io["wqkvf"], io["wo"], io["w1"], io["w2"]
    blob_f32, blob_bf16, rowblob = io["blob_f32"], io["blob_bf16"], io["rowblob"]
    b1col = io["b1col"]
    y_out, dump = io["y_out"], io["dump"]

    const = tc.alloc_tile_pool(name="const", bufs=1)
    dram = tc.alloc_tile_pool(name="dram", bufs=1, space="DRAM")

    # ---- warmup collective (prepay ncfw handshake) -----------------------
    wa_in = dram.tile([8, 128], BF16, name="wa_in")
    wa_out = dram.tile([8, 128], BF16, name="wa_out")
    nc.gpsimd.collective_compute("AllReduce", ALU.add, replica_groups=RG,
                                 ins=[wa_in.opt()], outs=[wa_out.opt()])

    a2a_in = dram.tile([128 * N_CORES, TAIL], BF16, name="a2a_in")
    a2a_out = dram.tile([128 * N_CORES, TAIL], BF16, name="a2a_out")

    # ---- constants into SBUF (SP queue) ---------------------------------
    cf = const.tile([128, 129], F32, name="cf")
    nc.sync.dma_start(cf[:], blob_f32.ap())
    ltriT_sb = cf[:, 0:128]
    onescol = cf[:, 128:129]                              # all ones
    cb = const.tile([128, 3, 128], BF16, name="cb")       # cmask|ident|bd128
    nc.sync.dma_start(cb[:], blob_bf16.ap().rearrange("p (n f) -> p n f", n=3))
    cmask_bc = cb[:, 0:1, :].to_broadcast([128, 4, 128])
    ident_sb, bd128_sb = cb[:, 1, :], cb[:, 2, :]
    rows = const.tile([1, 10, 128], BF16, name="rows")
    nc.sync.dma_start(rows[:], rowblob.ap().rearrange("o (n f) -> o n f", n=10))
    csq_r, csk_r, csv_r, csf_r = (rows[:, i, :] for i in range(4))
    ones_row = rows[:, 8, :]
    one1 = rows[:, 9, 0:1]
    bcol = const.tile([128, 4], F32, name="bcol")
    nc.sync.dma_start(bcol[:], io["bcols"].ap())
    b1c_sb = const.tile([128, MT], F32, name="b1c")
    nc.sync.dma_start(b1c_sb[:], b1col.ap())
    eps_sb = const.tile([128, 1], F32)
    nc.vector.memset(eps_sb[:], EPS)

    # ---- persistent activation tensors (allocated below xp/wp in the
    # pool stack; released only at the very end) --------------------------
    persist = tc.alloc_tile_pool(name="persist", bufs=1)
    qT = persist.tile([128, TOK], BF16, name="qT")
    kT = persist.tile([128, TOK], BF16, name="kT")
    k_tm = persist.tile([128, TT, 128], BF16, name="k_tm")
    v_tm = persist.tile([128, TT, 128], BF16, name="v_tm")
    sp = persist.tile([128, TT, 128], F32, name="sp")
    s_tm = persist.tile([128, TT, 128], BF16, name="s_tm")
    onT = persist.tile([128, TOK], BF16, name="onT")

    # ---- bulk loads -----------------------------------------------------
    xp = tc.alloc_tile_pool(name="xp", bufs=1)
    xT_sb = xp.tile([128, DT, TOK], BF16, name="xT_sb")
    for dt in range(DT):
        nc.sync.dma_start(xT_sb[:, dt, :],
                          xT.ap().rearrange("(dt p) t -> p dt t", p=P)[:, dt, :])
    wp = tc.alloc_tile_pool(name="wp", bufs=1)
    w4_sb = wp.tile([128, DT, 512], BF16, name="w4_sb")   # wq|wk|wv|wf cols
    nc.scalar.dma_start(w4_sb[:], wqkvf.ap().rearrange("(dt p) j -> p dt j", p=P))
    wo_sb = const.tile([128, DT, D], BF16, name="wo_sb")
    nc.scalar.dma_start(wo_sb[:], wo.ap().rearrange("(dt p) j -> p dt j", p=P))
    x2_sb = const.tile([128, 2, D], F32, name="x2_sb")
    nc.sync.dma_start(x2_sb[:], x_res.ap().rearrange("(n p) d -> p n d", p=P))

    # =====================================================================
    # P1: LN1 stats from xT via matmul reduction (per 512-token group)
    # =====================================================================
    stats = tc.alloc_tile_pool(name="stats", bufs=1)
    mu_r = stats.tile([1, 4, 512], BF16, name="mu_r")
    rstd_r = stats.tile([1, 4, 512], BF16, name="rstd_r")
    rbc = stats.tile([128, 4, 512], F32, name="rbc")
    oc = stats.tile([128, 1], BF16, name="oc")
    nc.vector.memset(oc[:], 1.0 / D)
    with tc.tile_pool(name="sq_p", bufs=3) as sqp, \
         tc.tile_pool(name="st_ps", bufs=2, space="PSUM") as stps, \
         tc.tile_pool(name="rb_ps", bufs=2, space="PSUM") as rbps, \
         tc.tile_pool(name="st_sb", bufs=2) as stsb:
        for g in range(4):
            gsl = slice(g * 512, (g + 1) * 512)
            ps_mu = stps.tile([1, 512], F32, name="ps_mu")
            ps_e2 = stps.tile([1, 512], F32, name="ps_e2")
            for dt in range(DT):
                sq = sqp.tile([128, 512], BF16, name="sq")
                nc.vector.tensor_tensor(sq[:], xT_sb[:, dt, gsl],
                                        xT_sb[:, dt, gsl], ALU.mult)
                nc.tensor.matmul(ps_mu[:], oc[:], xT_sb[:, dt, gsl],
                                 start=(dt == 0), stop=(dt == DT - 1))
                nc.tensor.matmul(ps_e2[:], oc[:], sq[:],
                                 start=(dt == 0), stop=(dt == DT - 1))
            # var = e2 - mu^2 ; rstd = exp(-0.5 ln(var+eps))
            nc.scalar.activation(mu_r[:, g, :], ps_mu[:], AF.Copy)
            var = stsb.tile([1, 512], F32, name="var")
            nc.vector.tensor_tensor(var[:], mu_r[:, g, :], mu_r[:, g, :],
                                    ALU.mult)
            nc.vector.tensor_tensor(var[:], ps_e2[:], var[:], ALU.subtract)
            lnv = stsb.tile([1, 512], F32, name="lnv")
            nc.scalar.activation(lnv[:], var[:], AF.Ln, bias=eps_sb[0:1, :])
            nc.scalar.activation(rstd_r[:, g, :], lnv[:], AF.Exp, scale=-0.5)
            ps_rb = rbps.tile([128, 512], F32, name="ps_rb")
            nc.tensor.matmul(ps_rb[:], ones_row, rstd_r[:, g, :],
                             start=True, stop=True)
            nc.vector.tensor_copy(rbc[:, g, :], ps_rb[:])

    # =====================================================================
    # P2: projections, feat-major, with rank-1 mean correction
    # =====================================================================
    ffeat = tc.alloc_tile_pool(name="ffeat", bufs=1)
    f_ft = ffeat.tile([128, TOK], BF16, name="f_ft")
    with tc.tile_pool(name="pj_ps", bufs=3, space="PSUM") as pjps, \
         tc.tile_pool(name="pj_sb", bufs=3) as pjsb, \
         tc.tile_pool(name="tr_ps", bufs=2, space="PSUM") as trps:
        def proj_psum(jsl, cs_row, g):
            gsl = slice(g * 512, (g + 1) * 512)
            bank = pjps.tile([128, 512], F32, name="pjbank")
            for dt in range(DT):
                nc.tensor.matmul(bank[:], w4_sb[:, dt, jsl], xT_sb[:, dt, gsl],
                                 start=(dt == 0), stop=False)
            nc.tensor.matmul(bank[:], cs_row, mu_r[:, g, :],
                             start=False, stop=True)
            return bank

        for g in range(4):                      # q: silu -> qT
            bank = proj_psum(slice(0, 128), csq_r, g)
            gsl = slice(g * 512, (g + 1) * 512)
            yp = pjsb.tile([128, 512], BF16, name="yp")
            nc.vector.tensor_tensor(yp[:], bank[:], rbc[:, g, :], ALU.mult)
            nc.scalar.activation(qT[:, gsl], yp[:], AF.Silu,
                                 bias=bcol[:, 0:1])
        for g in range(4):                      # k: silu -> kT (+ k_tm below)
            bank = proj_psum(slice(128, 256), csk_r, g)
            gsl = slice(g * 512, (g + 1) * 512)
            yp = pjsb.tile([128, 512], BF16, name="yp")
            nc.vector.tensor_tensor(yp[:], bank[:], rbc[:, g, :], ALU.mult)
            nc.scalar.activation(kT[:, gsl], yp[:], AF.Silu,
                                 bias=bcol[:, 1:2])
        for g in range(4):
            pst = trps.tile([128, 4, 128], BF16, name="pst")
            for cc in range(4):
                ti = g * 4 + cc
                nc.tensor.transpose(pst[:, cc, :],
                                    kT[:, ti * 128:(ti + 1) * 128], ident_sb)
            nc.vector.tensor_copy(k_tm[:, g * 4:(g + 1) * 4, :], pst[:])
        for g in range(4):                      # v: mult only -> v_tm
            bank = proj_psum(slice(256, 384), csv_r, g)
            yp = pjsb.tile([128, 512], BF16, name="yp")
            nc.vector.tensor_tensor(yp[:], bank[:], rbc[:, g, :], ALU.mult)
            pst = trps.tile([128, 4, 128], BF16, name="pst")
            for cc in range(4):
                nc.tensor.transpose(pst[:, cc, :], yp[:, cc * 128:(cc + 1) * 128],
                                    ident_sb)
            nc.vector.tensor_copy(v_tm[:, g * 4:(g + 1) * 4, :], pst[:])
        for g in range(4):                      # f -> f_ft (feat-major)
            bank = proj_psum(slice(384, 512), csf_r, g)
            gsl = slice(g * 512, (g + 1) * 512)
            nc.vector.tensor_tensor(f_ft[:, gsl], bank[:], rbc[:, g, :],
                                    ALU.mult)
        for g in range(4):                      # gates (ln/exp table)
            pst = trps.tile([128, 4, 128], BF16, name="pst")
            for cc in range(4):
                ti = g * 4 + cc
                nc.tensor.transpose(pst[:, cc, :],
                                    f_ft[:, ti * 128:(ti + 1) * 128], ident_sb)
            csl = slice(g * 4, (g + 1) * 4)
            enf = pjsb.tile([128, 4, 128], F32, name="enf")
            nc.scalar.activation(enf[:], pst[:], AF.Exp, scale=-1.0)
            nc.scalar.activation(sp[:, csl, :], enf[:], AF.Ln, bias=1.0)
            e8 = pjsb.tile([128, 4, 128], BF16, name="e8")
            nc.scalar.activation(e8[:], sp[:, csl, :], AF.Exp, scale=-0.125)
            nc.vector.tensor_scalar(s_tm[:, csl, :], e8[:], -1.0, 1.0,
                                    ALU.mult, ALU.add)
    ffeat.release()
    stats.release()
    wp.release()
    xp.release()

    # start w1 prefetch now (ACT queue), in mlp-dim chunks so MLP1 can
    # start consuming the first mt tiles as soon as they land
    w1p = tc.alloc_tile_pool(name="w1p", bufs=1)
    w1_sb = w1p.tile([128, DT, MLP], BF16, name="w1_sb")
    for q4 in range(4):
        msl = slice(q4 * 1024, (q4 + 1) * 1024)
        nc.scalar.dma_start(
            w1_sb[:, :, msl],
            w1.ap().rearrange("(dt p) m -> p dt m", p=P)[:, :, msl])

    for nm, t_sb in (("qT", qT), ("kT", kT)):
        if (d := dump(nm, [128, TOK], BF16)) is not None:
            nc.sync.dma_start(d.ap(), t_sb[:])
    for nm, t_sb in (("k_tm", k_tm), ("v_tm", v_tm), ("s_tm", s_tm)):
        if (d := dump(nm, [128, TT * 128], BF16)) is not None:
            nc.sync.dma_start(d.ap().rearrange("p (n f) -> p n f", n=TT), t_sb[:])
    if (d := dump("sp", [128, TT * 128])) is not None:
        nc.sync.dma_start(d.ap().rearrange("p (n f) -> p n f", n=TT), sp[:])

    # =====================================================================
    # P3: chunked scan.  Token tile index = b*4+c.  Group = fixed c, 4 b's.
    # =====================================================================
    scank = tc.alloc_tile_pool(name="scank", bufs=1)
    Kst = scank.tile([128, 4, 64], BF16, name="Kst")     # [(2h dk), b, m]
    Vst = scank.tile([128, 4, 64], BF16, name="Vst")     # [(2h m), b, dv]
    lam_a = scank.tile([128, NCH, 4, 128], BF16, name="lam_a")
    stil_a = scank.tile([128, NCH, 4, 128], BF16, name="stil_a")
    stT_a = scank.tile([128, NCH, 4, 128], BF16, name="stT_a")
    am_a = scank.tile([128, NCH, 2, 4, 128], BF16, name="am_a")
    dk_a = scank.tile([128, NCH, 4, 64], BF16, name="dk_a")   # [(2h dk), b, m]
    dv_a = scank.tile([128, NCH, 4, 64], BF16, name="dv_a")   # [(2h m), b, dv]
    lbc_a = scank.tile([128, NCH, 4, 128], BF16, name="lbc_a")
    dec_a = scank.tile([128, NCH, 4], BF16, name="dec_a")

    def cgv(t_sb, c):
        # [128, TT, f] -> [128, 4b, f] strided view for chunk c
        return t_sb[:].rearrange("p (b c) f -> p c b f", c=NCH)[:, c]

    # serial-phase psum pool FIRST so its banks are disjoint from prep's
    with tc.tile_pool(name="se_ps", bufs=1, space="PSUM") as seps, \
         tc.tile_pool(name="pr_ps", bufs=3, space="PSUM") as prps, \
         tc.tile_pool(name="pr_sb", bufs=2) as prsb, \
         tc.tile_pool(name="se_sb", bufs=2) as sesb:
        # ---------------- prep (state-independent), all c ----------------
        for c in range(NCH):
            ps_cs = prps.tile([128, 4, 128], F32, name="ps_cs", tag="pband")
            nc.tensor.matmul(ps_cs[:], ltriT_sb[:], cgv(sp, c),
                             start=True, stop=True)
            nc.scalar.activation(lam_a[:, c], ps_cs[:], AF.Exp)
            en4 = prsb.tile([128, 4, 128], BF16, name="en4")
            nc.scalar.activation(en4[:], ps_cs[:], AF.Exp, scale=-1.0)
            ps_ct = prps.tile([1, 4, 128], F32, name="ps_ct", tag="pband")
            nc.tensor.matmul(ps_ct[:], onescol, cgv(sp, c),
                             start=True, stop=True)
            lamCr = prsb.tile([1, 4, 128], BF16, name="lamCr")
            nc.scalar.activation(lamCr[:], ps_ct[:], AF.Exp, scale=-0.125)
            ps_lb = prps.tile([128, 4, 128], F32, name="ps_lb", tag="pband")
            nc.tensor.matmul(ps_lb[:], ones_row,
                             lamCr[:].rearrange("o b f -> o (b f)"),
                             start=True, stop=True)
            nc.scalar.activation(lbc_a[:, c], ps_lb[:], AF.Copy)
            nc.vector.tensor_tensor(stil_a[:, c], cgv(s_tm, c), en4[:],
                                    ALU.mult)
            s2 = prsb.tile([128, 4, 128], BF16, name="s2")
            nc.vector.tensor_tensor(s2[:], stil_a[:, c], ps_lb[:], ALU.mult)
            ps_dc = prps.tile([128, 4], F32, name="ps_dc", tag="pband")
            for b in range(4):
                nc.tensor.matmul(ps_dc[:, b:b + 1], lamCr[:, b, :], one1,
                                 start=True, stop=True)
            nc.scalar.activation(dec_a[:, c], ps_dc[:], AF.Copy)
            for h in range(2):
                hs = slice(h * 64, (h + 1) * 64)
                ps_a = prps.tile([128, 4, 128], F32, name="ps_a", tag="pband")
                for b in range(4):
                    tsl = slice((b * 4 + c) * 128, (b * 4 + c + 1) * 128)
                    nc.tensor.matmul(ps_a[:, b, :], kT[hs, tsl], qT[hs, tsl],
                                     start=True, stop=True)
                nc.vector.tensor_tensor(am_a[:, c, h], ps_a[:], cmask_bc,
                                        ALU.mult)
            ps_st = prps.tile([128, 4, 128], BF16, name="ps_st", tag="pband")
            for b in range(4):
                nc.tensor.transpose(ps_st[:, b, :], stil_a[:, c, b, :],
                                    ident_sb)
            nc.scalar.activation(stT_a[:, c], ps_st[:], AF.Copy)
            ps_dk = prps.tile([128, 4, 64], F32, name="ps_dk", tag="pband")
            for h in range(2):
                hs = slice(h * 64, (h + 1) * 64)
                for b in range(4):
                    bi = b * 4 + c
                    nc.tensor.matmul(ps_dk[hs, b, :], k_tm[:, bi, hs],
                                     s2[:, b, hs], start=True, stop=True)
            nc.scalar.activation(dk_a[:, c], ps_dk[:], AF.Copy)
            ps_dv = prps.tile([128, 4, 64], F32, name="ps_dv", tag="pband")
            for h in range(2):
                hs = slice(h * 64, (h + 1) * 64)
                for b in range(4):
                    bi = b * 4 + c
                    nc.tensor.matmul(ps_dv[hs, b, :], s2[:, b, hs],
                                     v_tm[:, bi, hs], start=True, stop=True)
            nc.scalar.activation(dv_a[:, c], ps_dv[:], AF.Copy)

        # ---------------- serial state chain -----------------------------
        for c in range(NCH):
            first = (c == 0)
            ps_ok = seps.tile([128, 4, 2, 64], F32, name="ps_ok")
            for h in range(2):
                hs = slice(h * 64, (h + 1) * 64)
                for b in range(4):
                    tsl = slice((b * 4 + c) * 128, (b * 4 + c + 1) * 128)
                    if not first:
                        nc.tensor.matmul(ps_ok[:, b, h, :], qT[hs, tsl],
                                         Kst[hs, b, :], start=True, stop=False)
                    nc.tensor.matmul(ps_ok[:, b, h, :], am_a[:, c, h, b, :],
                                     stil_a[:, c, b, hs],
                                     start=first, stop=True)
            oksc = sesb.tile([128, 4, 128], F32, name="oksc")
            nc.vector.tensor_tensor(
                oksc[:], ps_ok[:].rearrange("p b h f -> p b (h f)"),
                lam_a[:, c], ALU.mult)
            ex = sesb.tile([128, 4, 128], BF16, name="ex")
            nc.scalar.activation(ex[:], oksc[:], AF.Exp, scale=0.125)
            rsum = sesb.tile([128, 8], F32, name="rsum")
            nc.vector.tensor_reduce(
                rsum[:], ex[:].rearrange("p b (h s) -> p (b h) s", h=2),
                AX.X, ALU.add)
            rcp = sesb.tile([128, 8], F32, name="rcp")
            nc.vector.reciprocal(rcp[:], rsum[:])
            pl = sesb.tile([128, 4, 128], BF16, name="pl")
            nc.vector.tensor_tensor(pl[:], ex[:], lam_a[:, c], ALU.mult)
            nc.vector.tensor_tensor(
                pl[:].rearrange("p b (h s) -> p (b h) s", h=2),
                pl[:].rearrange("p b (h s) -> p (b h) s", h=2),
                rcp[:].rearrange("p (x o) -> p x o", x=8)
                .to_broadcast([128, 8, 64]),
                ALU.mult)
            ps_pt = seps.tile([128, 4, 128], BF16, name="ps_pt")
            for b in range(4):
                nc.tensor.transpose(ps_pt[:, b, :], pl[:, b, :], ident_sb)
            plT = sesb.tile([128, 4, 128], BF16, name="plT")
            nc.scalar.activation(plT[:], ps_pt[:], AF.Copy)
            ps_o = seps.tile([128, 4, 128], F32, name="ps_o")
            for h in range(2):
                hs = slice(h * 64, (h + 1) * 64)
                ps_b2 = seps.tile([128, 4, 128], F32, name="ps_b2")
                for b in range(4):
                    nc.tensor.matmul(ps_b2[:, b, :], stT_a[hs, c, b, :],
                                     plT[hs, b, :], start=True, stop=True)
                b2m = sesb.tile([128, 4, 128], BF16, name="b2m")
                nc.vector.tensor_tensor(b2m[:], ps_b2[:], cmask_bc, ALU.mult)
                for b in range(4):
                    bi = b * 4 + c
                    if not first:
                        nc.tensor.matmul(ps_o[hs, b, :], Vst[hs, b, :],
                                         plT[hs, b, :], start=True, stop=False)
                    nc.tensor.matmul(ps_o[hs, b, :], v_tm[:, bi, hs],
                                     b2m[:, b, :], start=first, stop=True)
            onv = onT[:].rearrange("p (b c f) -> p c b f", b=4, c=NCH)
            nc.scalar.activation(onv[:, c], ps_o[:], AF.Copy)
            if first:
                nc.vector.tensor_copy(Kst[:], dk_a[:, c])
                nc.vector.tensor_copy(Vst[:], dv_a[:, c])
            else:
                for h in range(2):
                    hs = slice(h * 64, (h + 1) * 64)
                    nc.vector.tensor_tensor(Kst[hs], Kst[hs],
                                            lbc_a[hs, c, :, hs], ALU.mult)
                nc.vector.tensor_tensor(Kst[:], Kst[:], dk_a[:, c], ALU.add)
                nc.vector.tensor_tensor(
                    Vst[:], Vst[:],
                    dec_a[:, c].rearrange("p (b o) -> p b o", b=4)
                    .to_broadcast([128, 4, 64]),
                    ALU.mult)
                nc.vector.tensor_tensor(Vst[:], Vst[:], dv_a[:, c], ALU.add)

    # batched RMS over dv for the whole onT
    with tc.tile_pool(name="rms_ps", bufs=2, space="PSUM") as rps, \
         tc.tile_pool(name="rms_sb", bufs=2) as rsb:
        for q4 in range(4):
            qsl = slice(q4 * 512, (q4 + 1) * 512)
            sqo = rsb.tile([128, 512], BF16, name="sqo")
            nc.vector.tensor_tensor(sqo[:], onT[:, qsl], onT[:, qsl], ALU.mult)
            ps_ss = rps.tile([128, 512], F32, name="ps_ss")
            nc.tensor.matmul(ps_ss[:], bd128_sb, sqo[:], start=True, stop=True)
            lns = rsb.tile([128, 512], F32, name="lns")
            nc.scalar.activation(lns[:], ps_ss[:], AF.Ln, bias=eps_sb[:],
                                 scale=1.0 / DV)
            rro = rsb.tile([128, 512], F32, name="rro")
            nc.scalar.activation(rro[:], lns[:], AF.Exp, scale=-0.5)
            nc.vector.tensor_tensor(onT[:, qsl], onT[:, qsl], rro[:], ALU.mult)

    if (d := dump("onT", [128, TOK], BF16)) is not None:
        nc.sync.dma_start(d.ap(), onT[:])

    # head-sharded -> token-sharded redistribution
    nc.sync.dma_start(
        a2a_in[:].rearrange("(r p) t -> p r t", p=P),
        onT[:].rearrange("p (r t) -> p r t", r=N_CORES))
    nc.gpsimd.collective_compute("AllToAll", ALU.bypass, replica_groups=RG,
                                 ins=[a2a_in.opt()], outs=[a2a_out.opt()])
    scank.release()

    # w2 prefetch (ACT queue): double-buffered chunks; first two issued
    # here, last two after MLP1 emission (their WAR deps are MLP2 reads,
    # which must already be behind them in the Act queue to avoid a
    # head-of-line deadlock)
    w2p = tc.alloc_tile_pool(name="w2p", bufs=2)
    w2_chunks = []

    def w2_chunk_dma(q4):
        w2t = w2p.tile([128, 8, D], BF16, name="w2t")
        nc.scalar.dma_start(
            w2t[:], w2.ap().rearrange("(n p) d -> p n d", p=P)
            [:, q4 * 8:(q4 + 1) * 8, :])
        w2_chunks.append(w2t)

    w2_chunk_dma(0)
    w2_chunk_dma(1)

    # =====================================================================
    # P4 tail: out-proj + residual + LN2 + MLP on 256 tokens
    # =====================================================================
    tkb = tc.alloc_tile_pool(name="tail_keep", bufs=1)
    ofT = tkb.tile([128, DT, TAIL], BF16, name="ofT")
    nc.sync.dma_start(ofT[:], a2a_out[:].rearrange("(jt p) t -> p jt t", p=P))
    h2 = tkb.tile([128, 2, D], BF16, name="h2")
    h2T = tkb.tile([128, DT, TAIL], BF16, name="h2T")
    zT = tkb.tile([128, MT, TAIL], BF16, name="zT")
    ys = tkb.tile([128, 2, D], F32, name="ys")

    with tc.tile_pool(name="op_ps", bufs=2, space="PSUM") as ops, \
         tc.tile_pool(name="tail_sb", bufs=2) as tsb:
        for tt2 in range(2):
            for nb in range(2):
                nsl = slice(nb * 512, (nb + 1) * 512)
                op_bank = ops.tile([128, 512], F32, name="op_bank")
                for jt in range(DT):
                    nc.tensor.matmul(op_bank[:],
                                     ofT[:, jt, tt2 * 128:(tt2 + 1) * 128],
                                     wo_sb[:, jt, nsl],
                                     start=(jt == 0), stop=(jt == DT - 1))
                nc.vector.tensor_tensor(x2_sb[:, tt2, nsl], op_bank[:],
                                        x2_sb[:, tt2, nsl], ALU.add)
        if (d := dump("x2", [128, 2 * D])) is not None:
            nc.sync.dma_start(d.ap().rearrange("p (n f) -> p n f", n=2),
                              x2_sb[:])

        # LN2 (token-major) -> h2 -> h2T via PE transposes
        ssum = tsb.tile([128, 2], F32, name="ssum")
        ssq = tsb.tile([128, 2], F32, name="ssq")
        sqd = tsb.tile([128, D], BF16, name="sqd")
        for tt2 in range(2):
            nc.vector.tensor_reduce(ssum[:, tt2:tt2 + 1], x2_sb[:, tt2, :],
                                    AX.X, ALU.add)
            nc.scalar.activation(sqd[:], x2_sb[:, tt2, :], AF.Square,
                                 accum_out=ssq[:, tt2:tt2 + 1])
        mu2 = tsb.tile([128, 2], F32, name="mu2")
        nc.vector.tensor_scalar_mul(mu2[:], ssum[:], 1.0 / D)
        var2 = tsb.tile([128, 2], F32, name="var2")
        nc.vector.tensor_tensor(var2[:], mu2[:], mu2[:], ALU.mult)
        ex22 = tsb.tile([128, 2], F32, name="ex22")
        nc.vector.tensor_scalar_mul(ex22[:], ssq[:], 1.0 / D)
        nc.vector.tensor_tensor(var2[:], ex22[:], var2[:], ALU.subtract)
        lnv2 = tsb.tile([128, 2], F32, name="lnv2")
        nc.scalar.activation(lnv2[:], var2[:], AF.Ln, bias=eps_sb[:])
        r2 = tsb.tile([128, 2], F32, name="r2")
        nc.scalar.activation(r2[:], lnv2[:], AF.Exp, scale=-0.5)
        nmu2 = tsb.tile([128, 2], F32, name="nmu2")
        nc.vector.tensor_tensor(nmu2[:], r2[:], mu2[:], ALU.mult)
        nc.vector.tensor_scalar_mul(nmu2[:], nmu2[:], -1.0)
        for tt2 in range(2):
            nc.scalar.activation(h2[:, tt2, :], x2_sb[:, tt2, :], AF.Identity,
                                 bias=nmu2[:, tt2:tt2 + 1],
                                 scale=r2[:, tt2:tt2 + 1])

    with tc.tile_pool(name="h2t_ps", bufs=2, space="PSUM") as hps:
        for dt in range(DT):
            ph = hps.tile([128, 2, 128], BF16, name="ph")
            for tt2 in range(2):
                nc.tensor.transpose(ph[:, tt2, :],
                                    h2[:, tt2, dt * 128:(dt + 1) * 128],
                                    ident_sb)
            nc.scalar.activation(h2T[:, dt, :], ph[:], AF.Copy)

    # MLP1 m-major: y1T[mt] = sum_dt w1[dt,mt]^T h2T[dt]; gelu -> zT
    with tc.tile_pool(name="y1_ps", bufs=4, space="PSUM") as y1ps:
        for mt in range(MT):
            y1b = y1ps.tile([128, TAIL], F32, name="y1b")
            msl = slice(mt * 128, (mt + 1) * 128)
            for dt in range(DT):
                nc.tensor.matmul(y1b[:], w1_sb[:, dt, msl], h2T[:, dt, :],
                                 start=(dt == 0), stop=(dt == DT - 1))
            nc.scalar.activation(zT[:, mt, :], y1b[:], AF.Gelu,
                                 bias=b1c_sb[:, mt:mt + 1])

    w2_chunk_dma(2)
    w2_chunk_dma(3)

    # MLP2: y2[t,:] = sum_mt zT[mt]^T w2[mt]
    with tc.tile_pool(name="y2_ps", bufs=1, space="PSUM") as y2ps:
        y2_banks = [y2ps.tile([128, 512], F32, name=f"y2b{i}")
                    for i in range(4)]
        for mt in range(MT):
            w2t = w2_chunks[mt // 8]
            for tt2 in range(2):
                for nb in range(2):
                    nc.tensor.matmul(
                        y2_banks[tt2 * 2 + nb],
                        zT[:, mt, tt2 * 128:(tt2 + 1) * 128],
                        w2t[:, mt % 8, nb * 512:(nb + 1) * 512],
                        start=(mt == 0), stop=(mt == MT - 1))
        for tt2 in range(2):
            for nb in range(2):
                nsl = slice(nb * 512, (nb + 1) * 512)
                nc.vector.tensor_tensor(ys[:, tt2, nsl],
                                        y2_banks[tt2 * 2 + nb],
                                        x2_sb[:, tt2, nsl], ALU.add)
    nc.sync.dma_start(y_out.ap().rearrange("(n p) d -> p n d", p=P), ys[:])

    for pool in (tkb, w2p, w1p, persist, dram, const):
        pool.release()


def _build():
    nc = bacc.Bacc("TRN2", target_bir_lowering=False, debug=False,
                   num_devices=N_CORES)

    def din(name, shape, dt=BF16):
        return nc.dram_tensor(name, shape, dt, kind="ExternalInput")

    io = dict(
        xT=din("xT", [D, TOK]),
        x_res=din("x_res", [TAIL, D], F32),
        wqkvf=din("wqkvf", [D, 512]),
        wo=din("wo", [D, D]),
        w1=din("w1", [D, MLP]),
        w2=din("w2", [MLP, D]),
        b1col=din("b1col", [128, MT], F32),
        bcols=din("bcols", [128, 4], F32),
        blob_f32=din("blob_f32", [128, 129], F32),
        blob_bf16=din("blob_bf16", [128, 3 * 128]),
        rowblob=din("rowblob", [1, 10 * 128]),
        y_out=nc.dram_tensor("y_out", [TAIL, D], F32, kind="ExternalOutput"),
    )

    dbg = [s for s in os.environ.get("GSA_DEBUG", "").split(",") if s]
    dbg_outs = {}

    def dump(name, shape, dt=F32):
        if name in dbg:
            t = nc.dram_tensor("dbg_" + name, shape, dt,
                               kind="ExternalOutput")
            dbg_outs[name] = t
            return t
        return None

    io["dump"] = dump
    with tile.TileContext(nc) as tcx:
        _emit(nc, tcx, io)
    nc.compile()
    return nc, sorted(dbg_outs)


def _host_prep(inputs):
    """Fold norms/biases into weights; build per-core in_maps."""
    f32 = np.float32
    bf16 = ml_dtypes.bfloat16
    x = np.asarray(inputs["hidden_states"], f32).reshape(TOK, D)
    ln1_w = np.asarray(inputs["ln1_w"], f32)
    ln1_b = np.asarray(inputs["ln1_b"], f32)
    ln2_w = np.asarray(inputs["ln2_w"], f32)
    ln2_b = np.asarray(inputs["ln2_b"], f32)
    gnorm = np.asarray(inputs["gnorm_w"], f32)
    Wq = np.asarray(inputs["Wq"], f32) * ln1_w[:, None]
    Wk = np.asarray(inputs["Wk"], f32) * ln1_w[:, None]
    Wv = np.asarray(inputs["Wv"], f32) * ln1_w[:, None]
    Wf = np.asarray(inputs["Wf"], f32) * ln1_w[:, None]
    bq = ln1_b @ np.asarray(inputs["Wq"], f32)
    bk = ln1_b @ np.asarray(inputs["Wk"], f32)
    bv = ln1_b @ np.asarray(inputs["Wv"], f32)
    bf_ = ln1_b @ np.asarray(inputs["Wf"], f32)
    assert np.allclose(bv, 0) and np.allclose(bf_, 0), \
        "v/f projection biases must be zero (ln1_b fold only done for q/k)"
    Wo = np.asarray(inputs["Wo"], f32) * np.tile(gnorm, H)[:, None]
    W1 = np.asarray(inputs["W1"], f32) * ln2_w[:, None]
    b1 = np.asarray(inputs["b1"], f32) + ln2_b @ np.asarray(inputs["W1"], f32)
    W2 = np.asarray(inputs["W2"], f32)
    b2 = np.asarray(inputs["b2"], f32)

    tri = np.tril(np.ones((128, 128), f32))  # [t, tau] tau<=t
    xT = np.ascontiguousarray(x.T.astype(bf16))

    common = dict(
        xT=xT,
        wo=np.ascontiguousarray(Wo.astype(bf16)),
        w1=np.ascontiguousarray(W1.astype(bf16)),
        w2=np.ascontiguousarray(W2.astype(bf16)),
        b1col=np.ascontiguousarray(b1.reshape(MT, 128).T.astype(f32)),
        blob_f32=np.ascontiguousarray(np.concatenate(
            [(-0.125 * tri).T, np.ones((128, 1), f32)], axis=1)),
        blob_bf16=np.ascontiguousarray(np.concatenate(
            [tri.T, np.eye(128, dtype=f32),
             np.kron(np.eye(2, dtype=f32), np.ones((64, 64), f32))],
            axis=1).astype(bf16)),
    )
    in_maps = []
    for r in range(N_CORES):
        jsl = slice(r * 128, (r + 1) * 128)  # 2 heads = 128 cols
        m = dict(common)
        m["x_res"] = np.ascontiguousarray(x[r * TAIL:(r + 1) * TAIL]
                                          + b2[None, :])
        wq, wk = Wq[:, jsl], Wk[:, jsl]
        wv, wf = Wv[:, jsl], Wf[:, jsl]
        m["wqkvf"] = np.ascontiguousarray(
            np.concatenate([wq, wk, wv, wf], axis=1).astype(bf16))
        rb = np.zeros((1, 10 * 128), f32)
        for i, w in enumerate((wq, wk, wv, wf)):
            rb[0, i * 128:(i + 1) * 128] = -w.sum(axis=0)
        rb[0, 8 * 128:9 * 128] = 1.0
        rb[0, 9 * 128] = 1.0
        m["rowblob"] = np.ascontiguousarray(rb.astype(bf16))
        bc = np.zeros((128, 4), f32)
        bc[:, 0], bc[:, 1] = bq[jsl], bk[jsl]
        m["bcols"] = np.ascontiguousarray(bc)
        in_maps.append(m)
    return in_maps


def kernel(**inputs):
    if "nc" not in _cache:
        _cache["nc"], _cache["dbg"] = _build()
    nc = _cache["nc"]
    in_maps = _host_prep(inputs)
    res = run_bass_kernel_spmd(nc, in_maps, core_ids=list(range(N_CORES)),
                               trace=bool(os.environ.get("GSA_TRACE")))
    _cache["last_results"] = res
    out = np.concatenate([res.results[r]["y_out"] for r in range(N_CORES)],
                         axis=0)
    return out.reshape(B, T, D)
